# revision 1
# baseline (speedup 1.0000x reference)
"""Multi-head attention (B=2, S=2048, D=1024, H=16, Hd=64) on 8 trn2 cores.

Sharding: batch x head-group. Core c handles batch c//4 and heads
[4*(c%4), 4*(c%4)+4). Each core computes its heads' Q/K/V projections
(column-parallel), the masked softmax attention for those heads, and a
row-parallel partial of the output projection. Host sums the 4 partials
per batch and adds the analytic bias terms (bv @ Wo.T + bo).

Mask: reference keeps the *upper* triangle (key >= query), so query q
attends keys k >= q; tiles strictly below the block diagonal are skipped.

Matmul operands are float16 (full PE rate, fp32 PSUM accumulation).
Softmax skips the max-subtraction: scores are qk/32 with |qk| < ~20, so
exp never overflows and softmax is shift-invariant.
"""

import contextlib

import os as _os

_jp = _os.environ.get("JAX_PLATFORMS", "")
if _jp and "axon" not in _jp:
    _os.environ["JAX_PLATFORMS"] = "axon," + _jp

import numpy as np

import concourse.bass as bass
import concourse.tile as tile
from concourse import bacc, mybir
from concourse.bass_utils import run_bass_kernel_spmd

F32 = mybir.dt.float32
F16 = mybir.dt.float16

B = 2
S = 2048
D = 1024
HD = 64
N_CORES = 8
HEADS_PER_CORE = 4
DSL = HEADS_PER_CORE * HD  # 256 projection columns per core
P = 128
NKC = S // P  # 16 key chunks
NST = S // P  # 16 seq tiles
NCC = D // P  # 8 contraction chunks
QCH = 512
NQC = S // QCH  # 4

SCALE = 1.0 / np.sqrt(np.float32(D))  # 1/32


def _f16(a: np.ndarray) -> np.ndarray:
    return np.ascontiguousarray(a, dtype=np.float16)


def _build_kernel(nc: bass.Bass, repeat: int = 1):
    xT = nc.dram_tensor("xT", (D, S), F16, kind="ExternalInput").ap()
    wqkvT = nc.dram_tensor("wqkvT", (D, 3 * DSL), F16, kind="ExternalInput").ap()
    woT = nc.dram_tensor("woT", (DSL, D), F16, kind="ExternalInput").ap()
    bqkT = nc.dram_tensor("bqkT", (DSL, 2), F32, kind="ExternalInput").ap()
    outp = nc.dram_tensor("outp", (S, D), F32, kind="ExternalOutput").ap()

    with tile.TileContext(nc) as tc:
        for _ in range(repeat):
            _emit(tc, nc, xT, wqkvT, woT, bqkT, outp)
    nc.compile()
    return nc


def _emit(tc, nc, xT, wqkvT, woT, bqkT, outp):
    ctx = contextlib.ExitStack()

    persist = ctx.enter_context(tc.tile_pool(name="persist", bufs=1))

    qt_sb = [persist.tile([P, S], F16, tag=f"qt{j}", name=f"qt{j}") for j in range(2)]
    kt_sb = [persist.tile([P, S], F16, tag=f"kt{j}", name=f"kt{j}") for j in range(2)]
    v_sb = [
        persist.tile([P, HEADS_PER_CORE, HD + 1], F16, tag=f"v{i}", name=f"v{i}")
        for i in range(NST)
    ]
    attnt_sb = [
        persist.tile([P, S], F16, tag=f"attnt{j}", name=f"attnt{j}") for j in range(2)
    ]
    rinvb_sb = [
        persist.tile([P, S], F32, tag=f"rinvb{j}", name=f"rinvb{j}") for j in range(2)
    ]
    # head h's 1/rowsum lives at partition 32*h (engine base-partition rule)
    rinv_sb = persist.tile([P, S], F32, tag="rinv", name="rinv")
    wot_sb = [
        persist.tile([P, D], F16, tag=f"wot{j}", name=f"wot{j}") for j in range(2)
    ]
    bias_sb = persist.tile([P, 2, 2], F32, tag="bias", name="bias")  # [d%128, j, proj]
    ones64_sb = persist.tile([P, HD], F32, tag="ones64", name="ones64")
    nc.vector.memset(ones64_sb[:], 1.0)

    dram_pool = ctx.enter_context(tc.tile_pool(name="dram", bufs=1, space="DRAM"))
    rinv_dram = dram_pool.tile([HEADS_PER_CORE, S], F32, tag="rinvd", name="rinvd")

    st_psum = ctx.enter_context(tc.tile_pool(name="st_psum", bufs=2, space="PSUM"))
    pv_psum = ctx.enter_context(tc.tile_pool(name="pv_psum", bufs=2, space="PSUM"))
    pt_pool = ctx.enter_context(tc.tile_pool(name="pt", bufs=12))

    out_pool = ctx.enter_context(tc.tile_pool(name="outp_sb", bufs=4))
    op_psum_cell = []

    def _outproj_sti(sti):
        ob = out_pool.tile([P, D], F32, tag="ob", name="ob")
        for e in range(2):
            op = op_psum_cell[0].tile([P, QCH], F32, tag="op", name="op")
            for j in range(2):
                nc.tensor.matmul(
                    op[:],
                    lhsT=attnt_sb[j][:, sti * P : (sti + 1) * P],
                    rhs=wot_sb[j][:, e * QCH : (e + 1) * QCH],
                    start=(j == 0),
                    stop=(j == 1),
                )
            esl = slice(e * QCH, (e + 1) * QCH)
            if (sti + e) % 2 == 0:
                nc.scalar.copy(ob[:, esl], op[:])
            else:
                nc.vector.tensor_copy(ob[:, esl], op[:])
            # store each half as soon as its eviction lands
            nc.sync.dma_start(
                out=outp[sti * P : (sti + 1) * P, esl], in_=ob[:, esl]
            )

    def _attn_g(hp, g, interleave=None):
        # one (head-pair, q-chunk) unit; local heads 2*hp, 2*hp+1
        if True:
            kjs = list(range(NKC - 1, 4 * g - 1, -1))  # descending
            pv = [
                pv_psum.tile([HD + 1, QCH], F32, tag="pv", name=f"pv{h}")
                for h in range(2)
            ]
            for kp in range(len(kjs) // 2):
                kj0, kj1 = kjs[2 * kp], kjs[2 * kp + 1]
                if interleave is not None:
                    interleave(kp)
                diag = kj1 - 4 * g <= 3  # pair inside the block-diagonal
                # kept q-prefix width per kj: query q attends keys k >= q,
                # so key chunk kj only matters for q < 128*(kj+1)
                wid = {
                    kj: (P * (kj - 4 * g + 1) if diag else QCH)
                    for kj in (kj0, kj1)
                }
                # S^T = K @ Q^T, both heads interleaved: the 64-row
                # contractions of heads 0/1 sit in PE row groups 0/1 and
                # overlap in the array (tile_position from base partition).
                stp = [
                    st_psum.tile([P, 2 * QCH], F32, tag="st", name=f"stp{h}")
                    for h in range(2)
                ]
                for i, kj in ((0, kj0), (1, kj1)):
                    for h in range(2):
                        row = slice(HD * h, HD * (h + 1))
                        nc.tensor.matmul(
                            stp[h][:, i * QCH : i * QCH + wid[kj]],
                            lhsT=kt_sb[hp][row, kj * P : (kj + 1) * P],
                            rhs=qt_sb[hp][row, g * QCH : g * QCH + wid[kj]],
                            start=True,
                            stop=True,
                        )
                for h in range(2):
                    pt = pt_pool.tile([P, 2 * QCH], F16, tag="pt", name="pt")
                    if diag:
                        for i, kj in ((0, kj0), (1, kj1)):
                            w = wid[kj]
                            nc.scalar.activation(
                                pt[:, i * QCH : i * QCH + w],
                                stp[h][:, i * QCH : i * QCH + w],
                                mybir.ActivationFunctionType.Exp,
                                scale=float(SCALE),
                            )
                            # triangle at the last 128 kept columns:
                            # keep iff p >= f_local
                            tri = slice(i * QCH + w - P, i * QCH + w)
                            nc.gpsimd.affine_select(
                                out=pt[:, tri],
                                in_=pt[:, tri],
                                compare_op=mybir.AluOpType.is_ge,
                                fill=0.0,
                                base=0,
                                channel_multiplier=1,
                                pattern=[[-1, P]],
                            )
                    else:
                        nc.scalar.activation(
                            pt[:], stp[h][:], mybir.ActivationFunctionType.Exp,
                            scale=float(SCALE),
                        )
                    hc = 2 * hp + h
                    for i, kj in ((0, kj0), (1, kj1)):
                        # kj=15 always has width 512: it covers the full
                        # PSUM bank as the start=True matmul; later
                        # (narrower) matmuls accumulate on covered bytes.
                        nc.tensor.matmul(
                            pv[h][:, 0 : wid[kj]],
                            lhsT=v_sb[kj][:, hc, :],
                            rhs=pt[:, i * QCH : i * QCH + wid[kj]],
                            start=(kj == kjs[0]),
                            stop=(kj == kjs[-1]),
                        )
            gsl = slice(g * QCH, (g + 1) * QCH)
            tail = hp == 1 and g == NQC - 1
            opb = (
                op_psum_cell[0].tile([P, QCH], F32, tag="op", name="opb")
                if tail
                else None
            )
            for h in range(2):
                nc.vector.tensor_copy(
                    attnt_sb[hp][HD * h : HD * (h + 1), gsl],
                    pv[h][0:HD, :],
                )
                hc = 2 * hp + h
                nc.vector.reciprocal(
                    out=rinv_sb[32 * hc : 32 * hc + 1, gsl],
                    in_=pv[h][HD : HD + 1, :],
                )
                # normalize attnT chunk by broadcast(1/r): SBUF APs cannot
                # have a zero partition step, so bounce through DRAM and
                # broadcast on the DRAM->SBUF read. (gpsimd
                # partition_broadcast passes CoreSim from non-zero base
                # partitions but produces wrong results on hardware.) For
                # the kernel-tail unit, broadcast via a rank-1 PE matmul
                # into PSUM instead: PE is idle there and the chain is
                # ~1.5us shorter than two DMA hops.
                if tail:
                    # base partition 96 exceeds the auto-derive cap; pass
                    # tile_position (row group, out col group) explicitly
                    nc.tensor.matmul(
                        opb[HD * h : HD * (h + 1), :],
                        lhsT=ones64_sb[32 * hc : 32 * hc + 1, :],
                        rhs=rinv_sb[32 * hc : 32 * hc + 1, gsl],
                        start=True,
                        stop=True,
                        tile_position=(32 * hc, HD * h),
                    )
                else:
                    nc.sync.dma_start(
                        out=rinv_dram[hc : hc + 1, gsl],
                        in_=rinv_sb[32 * hc : 32 * hc + 1, gsl],
                    )
                    rsrc = rinv_dram[hc : hc + 1, gsl]
                    bcast = bass.AP(
                        tensor=rsrc.tensor,
                        offset=rsrc.offset,
                        ap=[[0, HD]] + [list(p) for p in rsrc.ap[1:]],
                    )
                    nc.sync.dma_start(
                        out=rinvb_sb[hp][HD * h : HD * (h + 1), gsl], in_=bcast
                    )
            if tail:
                nc.vector.tensor_mul(
                    attnt_sb[hp][:, gsl], attnt_sb[hp][:, gsl], opb[:]
                )
            else:
                nc.vector.tensor_mul(
                    attnt_sb[hp][:, gsl], attnt_sb[hp][:, gsl], rinvb_sb[hp][:, gsl]
                )


    # --- phase 1: projections --------------------------------------------
    with tc.tile_pool(name="xw", bufs=1) as xw_pool:
        xt_t = [
            xw_pool.tile([P, S], F16, tag=f"xt{c}", name=f"xt{c}") for c in range(NCC)
        ]
        w_t = [
            xw_pool.tile([P, 3 * DSL], F16, tag=f"w{c}", name=f"w{c}")
            for c in range(NCC)
        ]
        for c in range(NCC):
            nc.sync.dma_start(out=w_t[c][:], in_=wqkvT[c * P : (c + 1) * P, :])
            nc.sync.dma_start(
                out=xt_t[c][:, 0:QCH], in_=xT[c * P : (c + 1) * P, 0:QCH]
            )
        # bqkT is (DSL, 2) = (d, proj); load as (128, j, proj).
        # Emitted after the first-group loads, before the bulk of x.
        nc.sync.dma_start(
            out=bias_sb[:],
            in_=bqkT.rearrange("(j p) t -> p j t", j=2),
        )
        for sch in range(1, NQC):
            for c in range(NCC):
                nc.sync.dma_start(
                    out=xt_t[c][:, sch * QCH : (sch + 1) * QCH],
                    in_=xT[c * P : (c + 1) * P, sch * QCH : (sch + 1) * QCH],
                )
        for j in range(2):
            nc.sync.dma_start(out=wot_sb[j][:], in_=woT[j * P : (j + 1) * P, :])


        with tc.tile_pool(name="proj_psum", bufs=2, space="PSUM") as proj_psum:
            def qk_proj(proj, j, sch_order=None):
                # QT/KT in transposed layout (d on partitions, seq on free)
                dst = qt_sb if proj == 0 else kt_sb
                woff = proj * DSL + j * P
                for sch in (sch_order or range(NQC)):
                    ps = proj_psum.tile([P, QCH], F32, tag="pp", name="pp")
                    for c in range(NCC):
                        nc.tensor.matmul(
                            ps[:],
                            lhsT=w_t[c][:, woff : woff + P],
                            rhs=xt_t[c][:, sch * QCH : (sch + 1) * QCH],
                            start=(c == 0),
                            stop=(c == NCC - 1),
                        )
                    # bias add (per-partition) + fp16 cast on eviction
                    nc.vector.tensor_scalar_add(
                        dst[j][:, sch * QCH : (sch + 1) * QCH],
                        ps[:],
                        bias_sb[:, j, proj : proj + 1],
                    )

            def v_proj(st):
                # V in natural layout (seq on partitions) + ones column
                ps = proj_psum.tile([P, DSL], F32, tag="pp", name="ppv")
                for c in range(NCC):
                    nc.tensor.matmul(
                        ps[:],
                        lhsT=xt_t[c][:, st * P : (st + 1) * P],
                        rhs=w_t[c][:, 2 * DSL : 3 * DSL],
                        start=(c == 0),
                        stop=(c == NCC - 1),
                    )
                nc.vector.tensor_copy(
                    v_sb[st][:, :, 0:HD],
                    ps[:].rearrange("p (h d) -> p h d", h=HEADS_PER_CORE),
                )
                nc.vector.memset(v_sb[st][:, :, HD : HD + 1], 1.0)

            # Attention for a head pair is emitted right after its QK
            # chunks so ScalarE exp work overlaps the remaining
            # projections; V projections interleave with (hp0, g0) in
            # program order (each PV follows its V tile's write).
            qk_proj(0, 0)
            qk_proj(1, 0)

            def emit_v(kp):
                v_proj(NST - 1 - 2 * kp)
                v_proj(NST - 2 - 2 * kp)

            _attn_g(0, 0, interleave=emit_v)
            for g in range(1, NQC):
                _attn_g(0, g)
            # hp1's attention consumes kt[1] from the highest key chunk
            # down (descending kj) but qt[1] from the lowest q chunk up:
            # produce them in first-use order.
            qk_proj(0, 1)
            qk_proj(1, 1, sch_order=range(NQC - 1, -1, -1))

    # projection pools closed: 2 PSUM banks free for the output projection
    op_psum_cell.append(
        ctx.enter_context(tc.tile_pool(name="op_psum", bufs=2, space="PSUM"))
    )
    for g in range(NQC):
        _attn_g(1, g)
        if g >= 1:
            for sti in range(4 * (g - 1), 4 * g):
                _outproj_sti(sti)
    for sti in range(4 * (NQC - 1), 4 * NQC):
        _outproj_sti(sti)

    ctx.close()


_NC_CACHE = None


def _get_nc():
    global _NC_CACHE
    if _NC_CACHE is None:
        nc = bacc.Bacc("TRN2", target_bir_lowering=False, debug=False)
        _NC_CACHE = _build_kernel(nc)
    return _NC_CACHE


def kernel(x, Wq, bq, Wk, bk, Wv, bv, Wo, bo):
    x = np.asarray(x, dtype=np.float32)
    Wq, bq = np.asarray(Wq, np.float32), np.asarray(bq, np.float32)
    Wk, bk = np.asarray(Wk, np.float32), np.asarray(bk, np.float32)
    Wv, bv = np.asarray(Wv, np.float32), np.asarray(bv, np.float32)
    Wo, bo = np.asarray(Wo, np.float32), np.asarray(bo, np.float32)

    nc = _get_nc()

    in_maps = []
    for c in range(N_CORES):
        b = c // 4
        hg = c % 4
        hsl = slice(hg * DSL, (hg + 1) * DSL)
        in_maps.append(
            {
                "xT": _f16(x[b].T),
                "wqkvT": _f16(
                    np.concatenate([Wq[hsl].T, Wk[hsl].T, Wv[hsl].T], axis=1)
                ),
                "woT": _f16(Wo[:, hsl].T),
                "bqkT": np.ascontiguousarray(
                    np.stack([bq[hsl], bk[hsl]], axis=1), dtype=np.float32
                ),
            }
        )

    res = run_bass_kernel_spmd(
        nc, in_maps, core_ids=list(range(N_CORES)), trace=False
    )

    # host gather: sum partials per batch, add analytic bias terms
    bias_term = (bv @ Wo.T + bo).astype(np.float32)  # (D,)
    out = np.empty((B, S, D), dtype=np.float32)
    for b in range(B):
        acc = res.results[4 * b]["outp"].astype(np.float32).copy()
        for c in range(4 * b + 1, 4 * b + 4):
            acc += res.results[c]["outp"]
        out[b] = acc + bias_term
    return out



# revision 13
# speedup vs baseline: 1.3520x; 1.3520x over previous
"""Multi-head attention (B=2, S=2048, D=1024, H=16, Hd=64) on 8 trn2 cores.

Sharding: batch x head-group. Core c handles batch c//4 and heads
[4*(c%4), 4*(c%4)+4).

Numerics (validated in numpy against the fp32 reference, rel ~4e-3):
- Q/K projections: fp8e4m3 x and fp8 weights (pre-scaled by 32 to avoid
  the fp8 subnormal region), DoubleRow perf mode: each matmul contracts
  256 rows at 0.5 cycles/output-column.
- V projection: compensated fp8 (x8*Wv8 + x8*rv8 + r8*Wv8 where r8/rv8
  are fp8 residuals) - plain fp8 V fails the 2e-2 gate, fp16 costs 2x.
- Scores: fp16 Q/K operands (64-dim contraction per head).
- Softmax: exp on ScalarE with scale 1/(32*32*32) (weight prescale
  folded in); no max-subtraction needed (|score| < ~0.4).
- PV: flipped layout - P^T tiles become the stationary operand so the
  output lands as [query-partitions, head-dim]; the ones-column of V
  gives the softmax row-sum per query partition, so normalization is a
  per-partition reciprocal + tensor_scalar multiply (no DRAM bounce).
- Attention outputs are transposed back to [dl, q] with PE transposes
  (identity matmul) for the fp16 output projection; partial outputs are
  stored fp16 and summed on the host (bv @ Wo.T + bo added analytically).

Mask: reference keeps the *upper* triangle (key >= query): query q
attends keys k >= q; tiles strictly below the block diagonal are skipped.
Key-chunk pairs are processed ascending so V tiles and KT chunks are
consumed in DMA-arrival order; diagonal pairs pack the second key
chunk's scores adjacent to the first so one exp covers both.

PSUM budget (8 banks): scores 2x[128,1024] (4) + PV accumulators
[128,1024] x1 (2) + outproj/transpose shared pool (2). PV packs 8
(head, q-chunk) accumulators of 65 fp32 into 2 banks; a cheap
zero-outer-product matmul pre-zeroes each bank so the packed slots can
all accumulate with start=False (PSUM start=True zeroing is
2KB-region-granular).
"""

import contextlib

import os as _os

_jp = _os.environ.get("JAX_PLATFORMS", "")
if _jp and "axon" not in _jp:
    _os.environ["JAX_PLATFORMS"] = "axon," + _jp

import numpy as np
import ml_dtypes

import concourse.bass as bass
import concourse.tile as tile
from concourse import bacc, mybir
from concourse.bass_utils import run_bass_kernel_spmd

F32 = mybir.dt.float32
F16 = mybir.dt.float16
F8 = mybir.dt.float8e4
NP8 = ml_dtypes.float8_e4m3

B = 2
S = 2048
D = 1024
HD = 64
N_CORES = 8
HPC = 4  # heads per core
DSL = HPC * HD  # 256 projection columns per core
P = 128
NST = S // P  # 16 seq tiles
QCH = 512
NQC = S // QCH  # 4

WS = 32.0  # weight prescale (keeps fp8 weights out of the subnormal range)
ESCALE = 1.0 / (np.sqrt(np.float32(D)) * WS * WS)  # exp scale: 1/32768

DR = mybir.MatmulPerfMode.DoubleRow
EXP = mybir.ActivationFunctionType.Exp


def _build_kernel(nc: bass.Bass, repeat: int = 1):
    x8d = nc.dram_tensor("x8p", (P, 4, 2, S), F8, kind="ExternalInput").ap()
    xr8d = nc.dram_tensor("xr8p", (P, 4, 2, S), F8, kind="ExternalInput").ap()
    w8d = nc.dram_tensor("w8p", (P, 4, 2, 3 * DSL), F8, kind="ExternalInput").ap()
    wvr8d = nc.dram_tensor("wvr8p", (P, 4, 2, DSL), F8, kind="ExternalInput").ap()
    wod = nc.dram_tensor("woT", (2, P, D), F16, kind="ExternalInput").ap()
    bqkd = nc.dram_tensor("bqk", (P, 2, 2), F32, kind="ExternalInput").ap()
    identd = nc.dram_tensor("ident", (P, P), F16, kind="ExternalInput").ap()
    outp = nc.dram_tensor("outp", (S, D), F16, kind="ExternalOutput").ap()

    with tile.TileContext(nc) as tc:
        for _ in range(repeat):
            _emit(tc, nc, x8d, xr8d, w8d, wvr8d, wod, bqkd, identd, outp)
    nc.compile()
    return nc


def _emit(tc, nc, x8d, xr8d, w8d, wvr8d, wod, bqkd, identd, outp):
    ctx = contextlib.ExitStack()

    persist = ctx.enter_context(tc.tile_pool(name="persist", bufs=1))

    x8_t = persist.tile([P, 4, 2, S], F8, tag="x8", name="x8")
    xr8_t = persist.tile([P, 4, 2, S], F8, tag="xr8", name="xr8")
    w8_t = persist.tile([P, 4, 2, 3 * DSL], F8, tag="w8", name="w8")
    wvr8_t = persist.tile([P, 4, 2, DSL], F8, tag="wvr", name="wvr")
    wo_t = [persist.tile([P, D], F16, tag=f"wo{j}", name=f"wo{j}") for j in range(2)]
    bias_sb = persist.tile([P, 2, 2], F32, tag="bias", name="bias")
    ident_sb = persist.tile([P, P], F16, tag="ident", name="ident")
    zeros_sb = persist.tile([P, QCH], F16, tag="zeros", name="zeros")
    qt_sb = [persist.tile([P, S], F8, tag=f"qt{j}", name=f"qt{j}") for j in range(2)]
    kt_sb = [persist.tile([P, S], F8, tag=f"kt{j}", name=f"kt{j}") for j in range(2)]
    q8_sb = [persist.tile([P, 2, S], F8, tag=f"q8{j}", name=f"q8{j}") for j in range(2)]
    k8_sb = [persist.tile([P, 2, S], F8, tag=f"k8{j}", name=f"k8{j}") for j in range(2)]
    v_sb = [
        persist.tile([P, HPC, HD + 1], F16, tag=f"v{i}", name=f"v{i}")
        for i in range(NST)
    ]
    attn_sb = [
        persist.tile([P, NST, P], F16, tag=f"attn{j}", name=f"attn{j}")
        for j in range(2)
    ]
    attnt_sb = [
        persist.tile([P, S], F16, tag=f"attnt{j}", name=f"attnt{j}") for j in range(2)
    ]
    rinv_sb = [
        persist.tile([P, NST, 2], F32, tag=f"rinv{j}", name=f"rinv{j}")
        for j in range(2)
    ]

    nc.vector.memset(zeros_sb[:], 0.0)

    # --- input DMAs (SP queue; s-quartered so early seq chunks arrive first;
    # Q/K weight columns before V so the first score tile is unblocked ASAP)
    nc.sync.dma_start(out=bias_sb[:], in_=bqkd)
    nc.sync.dma_start(out=x8_t[:, :, :, 0:QCH], in_=x8d[:, :, :, 0:QCH])
    nc.sync.dma_start(out=w8_t[:, :, :, 0 : 2 * DSL], in_=w8d[:, :, :, 0 : 2 * DSL])
    nc.sync.dma_start(out=xr8_t[:, :, :, 0:QCH], in_=xr8d[:, :, :, 0:QCH])
    nc.sync.dma_start(
        out=w8_t[:, :, :, 2 * DSL : 3 * DSL], in_=w8d[:, :, :, 2 * DSL : 3 * DSL]
    )
    nc.sync.dma_start(out=wvr8_t[:], in_=wvr8d)
    nc.sync.dma_start(out=ident_sb[:], in_=identd)
    for q4 in range(1, 4):
        ssl = slice(q4 * QCH, (q4 + 1) * QCH)
        nc.sync.dma_start(out=x8_t[:, :, :, ssl], in_=x8d[:, :, :, ssl])
        nc.sync.dma_start(out=xr8_t[:, :, :, ssl], in_=xr8d[:, :, :, ssl])
    for j in range(2):
        nc.sync.dma_start(out=wo_t[j][:], in_=wod[j])

    st_pool = ctx.enter_context(tc.tile_pool(name="st_psum", bufs=2, space="PSUM"))
    pv_pool = ctx.enter_context(tc.tile_pool(name="pv_psum", bufs=1, space="PSUM"))
    pt_pool = ctx.enter_context(tc.tile_pool(name="pt", bufs=6))
    ob_pool = ctx.enter_context(tc.tile_pool(name="ob", bufs=4))
    op_psum_cell = []

    # PV accumulator slot: m = h*4 + ql; slots 0-6 packed in bank 0,
    # slot 7 at the start of bank 1 (matmul outputs may not straddle banks).
    def pv_slot(pv, h, ql):
        m = h * 4 + ql
        if m < 7:
            return pv[:, 65 * m : 65 * m + 65]
        return pv[:, 512 : 512 + 65]

    def _transp(hp, qq):
        tp = op_psum_cell[0].tile([P, P], F16, tag="op", name="tp")
        nc.tensor.transpose(tp[:], attn_sb[hp][:, qq, :], ident_sb[:])
        if hp == 1 and qq >= 12:
            nc.scalar.copy(attnt_sb[hp][:, qq * P : (qq + 1) * P], tp[:])
        else:
            nc.vector.tensor_copy(attnt_sb[hp][:, qq * P : (qq + 1) * P], tp[:])

    ob_tiles = {}

    def outproj_unit(sti, e, slotkind="op"):
        if e == 0:
            ob_tiles[sti] = ob_pool.tile([P, D], F16, tag="ob", name="ob")
        ob = ob_tiles[sti]
        if slotkind == "st":
            # scores are done: borrow a free st-pool bank for extra slots
            stt = st_pool.tile([P, 1024], F32, tag="st", name="sttail")
            op = stt[:, e * QCH : (e + 1) * QCH]
        elif slotkind == "pv":
            pvt = pv_pool.tile([P, 1024], F32, tag="pv", name="pvtail")
            op = pvt[:, e * QCH : (e + 1) * QCH]
        else:
            op = op_psum_cell[0].tile([P, QCH], F32, tag="op", name="op")[:]
        for j in range(2):
            nc.tensor.matmul(
                op,
                lhsT=attnt_sb[j][:, sti * P : (sti + 1) * P],
                rhs=wo_t[j][:, e * QCH : (e + 1) * QCH],
                start=(j == 0),
                stop=(j == 1),
                skip_group_check=(slotkind == "pv"),
            )
        esl = slice(e * QCH, (e + 1) * QCH)
        # GPSIMD cannot read PSUM: evictions go to DVE; late ones to the
        # mostly-idle ScalarE
        if sti >= 8 and e == 0:
            nc.scalar.copy(ob[:, esl], op)
        else:
            nc.vector.tensor_copy(ob[:, esl], op)
        if e == 1:
            nc.sync.dma_start(out=outp[sti * P : (sti + 1) * P, :], in_=ob[:])

    def _attn(hp, g, interleave=None):
        kjs = list(range(4 * g, NST))  # ascending
        pairs = [(kjs[2 * i], kjs[2 * i + 1]) for i in range(len(kjs) // 2)]
        pv = pv_pool.tile([P, 1024], F32, tag="pv", name="pv")
        zeroed = []

        def zero_banks():
            # pre-zero both banks so packed slots accumulate with start=False.
            # Deferred past the first scores so the WAR on the previous g's
            # normalize doesn't block the PE head.
            nc.tensor.matmul(
                pv[:, 0:455], lhsT=zeros_sb[:, 0:P], rhs=zeros_sb[:, 0:455],
                start=True, stop=True, skip_group_check=True,
            )
            nc.tensor.matmul(
                pv[:, 512 : 512 + 65], lhsT=zeros_sb[:, 0:P], rhs=zeros_sb[:, 0:65],
                start=True, stop=True, skip_group_check=True,
            )
            zeroed.append(True)

        def emit_pv(units):
            if not zeroed:
                zero_banks()
            for pt, kj, coff in units:
                for h in range(2):
                    hc = 2 * hp + h
                    for ql in range(min(kj - 4 * g, 3) + 1):
                        nc.tensor.matmul(
                            pv_slot(pv, h, ql),
                            lhsT=pt[h][:, coff + P * ql : coff + P * (ql + 1)],
                            rhs=v_sb[kj][:, hc, :],
                            start=False,
                            stop=(kj == NST - 1),
                            skip_group_check=True,
                        )

        prev = None
        for pi, (kj0, kj1) in enumerate(pairs):
            if interleave is not None:
                interleave(2 * pi)
            diag = kj1 - 4 * g <= 3
            wid = {
                kj: (P * (kj - 4 * g + 1) if kj - 4 * g <= 3 else QCH)
                for kj in (kj0, kj1)
            }
            # PSUM start=True zeroes a whole 2KB bank, and matmul outputs
            # must not straddle banks: wider chunk first at offset 0
            # (start=True); the narrower one either starts at the bank-1
            # boundary (start=True) or overwrites pending-zeroed bytes of
            # bank 0 (start=False).
            if diag:
                parts = [(kj1, 0, True), (kj0, wid[kj1], wid[kj1] == QCH)]
            else:
                parts = [(kj0, 0, True), (kj1, QCH, True)]
            tot = wid[kj0] + wid[kj1]
            pts = []
            for h in range(2):
                st = st_pool.tile([P, 1024], F32, tag="st", name="st")
                b32 = slice(32 * h, 32 * h + 32)
                for kj, coff, sflag in parts:
                    nc.tensor.matmul(
                        st[:, coff : coff + wid[kj]],
                        lhsT=k8_sb[hp][b32, :, kj * P : (kj + 1) * P],
                        rhs=q8_sb[hp][b32, :, g * QCH : g * QCH + wid[kj]],
                        start=sflag,
                        stop=True,
                        skip_group_check=not sflag,
                        perf_mode=DR,
                    )
                if prev is not None and h == 1:
                    emit_pv(prev)
                    prev = None
                pt = pt_pool.tile([P, 1024], F16, tag="pt", name="pt")
                nc.scalar.activation(
                    pt[:, 0:tot], st[:, 0:tot], EXP, scale=float(ESCALE)
                )
                if diag:
                    for kj, coff, _ in parts:
                        tri = slice(coff + wid[kj] - P, coff + wid[kj])
                        nc.gpsimd.affine_select(
                            out=pt[:, tri],
                            in_=pt[:, tri],
                            compare_op=mybir.AluOpType.is_ge,
                            fill=0.0,
                            base=0,
                            channel_multiplier=1,
                            pattern=[[-1, P]],
                        )
                pts.append(pt)
                if interleave is not None and h == 0:
                    interleave(2 * pi + 1)
            prev = [(pts, kj, coff) for kj, coff, _ in parts]
        emit_pv(prev)

        # normalize: per (head, q-subchunk): 1/rowsum (ones column) then
        # per-partition scale into attn_sb [q, (h, d)]
        for h in range(2):
            for ql in range(4):
                qq = 4 * g + ql
                sl = pv_slot(pv, h, ql)
                nc.vector.reciprocal(
                    out=rinv_sb[hp][:, qq, h : h + 1], in_=sl[:, HD : HD + 1]
                )
                nc.vector.tensor_scalar_mul(
                    attn_sb[hp][:, qq, HD * h : HD * (h + 1)],
                    sl[:, 0:HD],
                    rinv_sb[hp][:, qq, h : h + 1],
                )

    # --- phase 1: projections + hp0 attention ------------------------------
    with tc.tile_pool(name="proj_psum", bufs=2, space="PSUM") as pp:

        def qk_proj(proj, j, schs):
            dst = qt_sb if proj == 0 else kt_sb
            dst8 = q8_sb if proj == 0 else k8_sb
            woff = proj * DSL + j * P
            for sch in schs:
                ssl = slice(sch * QCH, (sch + 1) * QCH)
                ps = pp.tile([P, QCH], F32, tag="pp", name="pp")
                for c in range(4):
                    nc.tensor.matmul(
                        ps[:],
                        lhsT=w8_t[:, c, :, woff : woff + P],
                        rhs=x8_t[:, c, :, sch * QCH : (sch + 1) * QCH],
                        start=(c == 0),
                        stop=(c == 3),
                        perf_mode=DR,
                    )
                # bias add (per-partition, prescaled by 32) + fp8 cast
                nc.vector.tensor_scalar_add(dst[j][:, ssl], ps[:], bias_sb[:, j, proj : proj + 1])
                # partition remap into the DoubleRow score layout:
                # staging p=(hl i r) -> [32*(2j+hl)+r, i, s]
                for hl in range(2):
                    nc.sync.dma_start(
                        out=dst8[j][32 * hl : 32 * hl + 32, :, ssl].rearrange(
                            "r i s -> i r s"
                        ),
                        in_=dst[j][64 * hl : 64 * hl + 64, ssl].rearrange(
                            "(i r) s -> i r s", i=2
                        ),
                    )

        def v_proj(st):
            ps = pp.tile([P, DSL], F32, tag="pp", name="ppv")
            ssl = slice(st * P, (st + 1) * P)
            vw = slice(2 * DSL, 3 * DSL)
            k = 0
            for lhs_t, rhs_t, rsl in (
                (x8_t, w8_t, vw),
                (x8_t, wvr8_t, slice(0, DSL)),
                (xr8_t, w8_t, vw),
            ):
                for c in range(4):
                    nc.tensor.matmul(
                        ps[:],
                        lhsT=lhs_t[:, c, :, ssl],
                        rhs=rhs_t[:, c, :, rsl],
                        start=(k == 0),
                        stop=(k == 11),
                        perf_mode=DR,
                    )
                    k += 1
            # descale (1/32) + fp16 cast
            nc.vector.tensor_scalar_mul(
                v_sb[st][:, :, 0:HD],
                ps[:].rearrange("p (h d) -> p h d", h=HPC),
                1.0 / WS,
            )
            nc.vector.memset(v_sb[st][:, :, HD : HD + 1], 1.0)

        qk_proj(0, 0, [0])
        qk_proj(1, 0, [0])

        # units interleaved into attn(0,0): v tiles ascending + remaining
        # j0 projection chunks in first-use order
        a00_units = {
            2: [("k", 0, 1)],
            4: [("q", 0, 1)],
            6: [("k", 0, 2)],
            8: [("q", 0, 2)],
            10: [("k", 0, 3)],
            12: [("q", 0, 3)],
        }

        def emit_a00(hi):
            if hi >= 2:
                v_proj(hi - 2)
            if hi == NST - 1:
                v_proj(NST - 2)
                v_proj(NST - 1)
            for kind, j, s in a00_units.get(hi, ()):
                qk_proj(0 if kind == "q" else 1, j, [s])

        _attn(0, 0, interleave=emit_a00)

        j1_units = [("q", 1, s) for s in range(NQC)] + [
            ("k", 1, s) for s in range(NQC)
        ]

        def emit_j1(hi):
            if hi % 2 == 0 and j1_units:
                kind, j, s = j1_units.pop(0)
                qk_proj(0 if kind == "q" else 1, j, [s])

        _attn(0, 1, interleave=emit_j1)
        _attn(0, 2, interleave=emit_j1)
        _attn(0, 3)

    # projection pool closed: 2 PSUM banks free for outproj/transposes
    op_psum_cell.append(
        ctx.enter_context(tc.tile_pool(name="op_psum", bufs=2, space="PSUM"))
    )

    # interleave units for the hp1 phase: hp0 transposes during g=0, then
    # hp1 transposes + output projections as their inputs become ready.
    def make_units(g):
        units = []
        if g == 0:
            for qq in range(NST):
                units.append(("t", 0, qq))
        else:
            for qq in range(4 * (g - 1), 4 * g):
                units.append(("t", 1, qq))
            for sti in range(4 * (g - 1), 4 * g):
                units.append(("o", sti, 0))
                units.append(("o", sti, 1))
        return units

    for g in range(NQC):
        units = make_units(g)
        npts = NST - 4 * g  # interleave points (2 per pair)
        per = (len(units) + npts - 1) // npts

        def emit_units(hi, units=units, per=per):
            for _ in range(per):
                if units:
                    kind, a, b = units.pop(0)
                    if kind == "t":
                        _transp(a, b)
                    else:
                        outproj_unit(a, b)

        _attn(1, g, interleave=emit_units)
        while units:
            kind, a, b = units.pop(0)
            if kind == "t":
                _transp(a, b)
            else:
                outproj_unit(a, b)

    # tail: last transposes + output projections
    for qq in range(12, NST):
        _transp(1, qq)
    for sti, kind in ((12, "st"), (13, "op"), (14, "pv"), (15, "st")):
        outproj_unit(sti, 0, slotkind=kind)
        outproj_unit(sti, 1, slotkind=kind)

    ctx.close()


_NC_CACHE = None


def _get_nc():
    global _NC_CACHE
    if _NC_CACHE is None:
        nc = bacc.Bacc("TRN2", target_bir_lowering=False, debug=False)
        _NC_CACHE = _build_kernel(nc)
    return _NC_CACHE


def _pairs(a, cols):
    """[1024, cols] fp8 -> [128, 4, 2, cols] pair-interleaved layout:
    out[p, c, i, s] = a[256c + 128i + p, s]."""
    return np.ascontiguousarray(
        np.asarray(a, dtype=NP8).reshape(4, 2, P, cols).transpose(2, 0, 1, 3)
    )


def kernel(x, Wq, bq, Wk, bk, Wv, bv, Wo, bo):
    x = np.asarray(x, dtype=np.float32)
    Wq, bq = np.asarray(Wq, np.float32), np.asarray(bq, np.float32)
    Wk, bk = np.asarray(Wk, np.float32), np.asarray(bk, np.float32)
    Wv, bv = np.asarray(Wv, np.float32), np.asarray(bv, np.float32)
    Wo, bo = np.asarray(Wo, np.float32), np.asarray(bo, np.float32)

    nc = _get_nc()

    ident = np.eye(P, dtype=np.float16)
    in_maps = []
    for c in range(N_CORES):
        b = c // 4
        hg = c % 4
        hsl = slice(hg * DSL, (hg + 1) * DSL)
        xT = np.ascontiguousarray(x[b].T)  # [1024, 2048]
        x8 = np.asarray(xT, dtype=NP8)
        xr = xT - x8.astype(np.float32)
        wqkv = np.concatenate(
            [WS * Wq[hsl].T, WS * Wk[hsl].T, WS * Wv[hsl].T], axis=1
        )  # [1024, 768]
        w8 = np.asarray(wqkv, dtype=NP8)
        wvr = wqkv[:, 2 * DSL :] - w8[:, 2 * DSL :].astype(np.float32)
        bqk = np.stack([WS * bq[hsl], WS * bk[hsl]], axis=1).reshape(2, P, 2)
        bqk = np.ascontiguousarray(bqk.transpose(1, 0, 2), dtype=np.float32)
        in_maps.append(
            {
                "x8p": _pairs(x8, S),
                "xr8p": _pairs(xr, S),
                "w8p": _pairs(w8, 3 * DSL),
                "wvr8p": _pairs(wvr, DSL),
                "woT": np.ascontiguousarray(
                    Wo[:, hsl].T.reshape(2, P, D), dtype=np.float16
                ),
                "bqk": bqk,
                "ident": ident,
            }
        )

    res = run_bass_kernel_spmd(
        nc, in_maps, core_ids=list(range(N_CORES)), trace=False
    )

    # host gather: sum partials per batch, add analytic bias terms
    bias_term = (bv @ Wo.T + bo).astype(np.float32)  # (D,)
    out = np.empty((B, S, D), dtype=np.float32)
    for b in range(B):
        acc = res.results[4 * b]["outp"].astype(np.float32)
        for c in range(4 * b + 1, 4 * b + 4):
            acc = acc + res.results[c]["outp"].astype(np.float32)
        out[b] = acc + bias_term
    return out
